# revision 13
# baseline (speedup 1.0000x reference)
"""Trainium2 Bass kernel for nn_BlockCrossAttn (block-diagonal attention, E=H=1).

Math per (block b, batch n) pair (256-long vectors q', k', v of the block):
    q' = wq*Q + bq ; k' = wk*K + bk
    soft[q,k] = softmax_k(q'[q] * k'[k])
    out[q] = wvo * (sum_k soft[q,k] * V[k]) + (bvo + bo)

Sharding: 128 blocks of 256 rows; 16 blocks per core across 8 cores.

Per-core pipeline (512 pairs, 171 groups of <=3).  Scores S^T[k, (t,q)] are
PSUM-drain-bound on the PE (~0.9ns/col regardless of matmul shape), and exp
is ScalarE-bound (~0.96ns/col), so groups are split across three classes to
balance all four engines:

  - class A (default): one K=2 PE matmul per pair (host-packed zero-padded
    [2, 640] = [q|0|k0 ; 0|q|k1] fp16 operands) -> PSUM bank; ScalarE exp
    [128, 1536] -> E fp16 in SBUF.
  - class C (PE relief, g%8 in {1,3,5}): GpSimd partition-broadcasts the q
    row; VectorE tensor_scalar (per-partition k scalar AP) builds the score
    span in SBUF at 2-byte rate; ScalarE exp from SBUF.
  - class B (ScalarE relief, g%8==7): PE scores as in A, then VectorE
    computes a two-term Schraudolph exp E = f16^(i16(s*1024*log2e + B1)) +
    sqrt2 * f16^(i16(s*1024*log2e + B2)) (~0.8% rel err, common scale
    cancels in softmax).  The sqrt2 combine is folded into the reduction:
    the second term uses a sqrt2-scaled [ones,v_hi,v_lo] triplet and
    accumulates into the same PSUM region (4 matmuls instead of 2).

  - Reduction: lhsT = [ones, v_hi, v_lo] fp16 (host-packed), rhs = E; the
    k-half matmuls accumulate into one PSUM [3, 256] region, 8 pairs per
    result bank.  VectorE flushes banks; a DRAM scratch bounce re-lays 32
    pairs into [32, 768]; num = hi+lo, reciprocal_approx_fast, multiply,
    affine epilogue; one contiguous DMA per block to the n-major output.
"""

from contextlib import ExitStack

import numpy as np

import concourse.bacc as bacc
import concourse.bass as bass
import concourse.tile as tile
from concourse import mybir
from concourse.bass_utils import run_bass_kernel_spmd

FP = mybir.dt.float32
F16 = mybir.dt.float16
I16 = mybir.dt.int16
AF = mybir.ActivationFunctionType
ALU = mybir.AluOpType

L = 32768          # sequence length
N = 32             # batch
BS = 256           # block size
NB = L // BS       # 128 blocks
NCORES = 8
BPC = NB // NCORES  # 16 blocks per core
LS = BPC * BS       # 4096 rows per core shard

GROUP = 3           # pairs per PSUM stage group (3 banks)
PAIRS = BPC * N     # 512 pairs per core
PW = 640            # qz cols per pair: rhs 512 | lhsT 128
SPD = 16            # pairs per staging DMA

# Schraudolph constants (fp16-domain, int16-bitcast, +0.25 hedges the
# rint-vs-trunc convert ambiguity); E = a + sqrt2*b via the dual triplet.
SCHR_SCALE = float(1024.0 * np.log2(np.e))
SCHR_B1 = 15305.25
SCHR_B2 = 14792.25
SQRT2 = float(np.sqrt(2.0))


def group_class(g):
    if g % 2 == 1:
        return "C"
    if g % 16 == 8:
        return "B"
    return "A"


def build_kernel_module(sc, reps: int = 1) -> bass.Bass:
    nc = bacc.Bacc("TRN2", target_bir_lowering=False, debug=False, num_devices=NCORES)
    qz = nc.declare_dram_parameter("qz", [PAIRS, 2, PW], F16, isOutput=False)
    vz = nc.declare_dram_parameter("vz", [BPC, 128, 2 * N * 6], F16, isOutput=False)
    kz = nc.declare_dram_parameter("kz", [BPC, 128, 2 * N], FP, isOutput=False)
    out_t = nc.declare_dram_parameter("out_t", [N, LS], FP, isOutput=True)

    with tile.TileContext(nc) as tc:
        with ExitStack() as ctx:
            if reps == 1:
                _emit(ctx, tc, qz, vz, kz, out_t, sc)
            else:
                with tc.For_i(0, reps, 1):
                    _emit(ctx, tc, qz, vz, kz, out_t, sc)
    nc.compile()
    return nc


def _emit(ctx, tc, qz, vz, kz, out_t, sc):
    nc = tc.nc

    stage = ctx.enter_context(tc.tile_pool(name="stage", bufs=2))
    vpool = ctx.enter_context(tc.tile_pool(name="vpool", bufs=2))
    kpool = ctx.enter_context(tc.tile_pool(name="kpool", bufs=2))
    qpool = ctx.enter_context(tc.tile_pool(name="qpool", bufs=12))
    epool = ctx.enter_context(tc.tile_pool(name="epool", bufs=5))
    spool = ctx.enter_context(tc.tile_pool(name="spool", bufs=8))
    dpool = ctx.enter_context(tc.tile_pool(name="dpool", bufs=2))
    ps_stage = ctx.enter_context(tc.tile_pool(name="ps_stage", bufs=2, space="PSUM"))
    ps_res = ctx.enter_context(tc.tile_pool(name="ps_res", bufs=2, space="PSUM"))
    drs = ctx.enter_context(tc.tile_pool(name="drs", bufs=2, space="DRAM"))

    def load_qz(c):
        qk = stage.tile([2, SPD * PW], F16, name="qk", tag="qk")
        nc.sync.dma_start(
            out=qk[:].rearrange("p (s w) -> p s w", s=SPD),
            in_=qz[c * SPD:(c + 1) * SPD].rearrange("s p w -> p s w"),
        )
        return qk

    def load_vz(b):
        vc = vpool.tile([128, 2, N, 6], F16, name="vc", tag="vc")
        nc.sync.dma_start(out=vc[:].rearrange("p t n c -> p (t n c)"), in_=vz[b])
        kc = kpool.tile([128, 2, N], FP, name="kc", tag="kc")
        nc.sync.dma_start(out=kc[:].rearrange("p t n -> p (t n)"), in_=kz[b])
        return vc, kc

    # --- reduction / division ---------------------------------------------------
    res_state = {"tile": None, "count": 0, "nflush": 0, "rs": None, "first_g": 0}

    def emit_reduces(pend):
        rhs_tiles, members = pend
        for (s, b, n, vc) in members:
            g = b * N + n
            p8 = res_state["count"]
            if p8 == 0:
                res_state["tile"] = ps_res.tile([128, 512], FP, name="res", tag="res")
                if res_state["nflush"] == 0:
                    res_state["rs"] = dpool.tile([128, 2048], FP, name="rs", tag="rs")
                    res_state["first_g"] = g
            j, h = p8 % 4, p8 // 4
            nmm = len(rhs_tiles) * 2
            i = 0
            for (e, c0) in rhs_tiles:
                for t in (0, 1):
                    nc.tensor.matmul(
                        res_state["tile"][32 * j:32 * j + 3, h * 256:(h + 1) * 256],
                        lhsT=vc[:][:, t, n, c0:c0 + 3],
                        rhs=e[:][:, s * 512 + t * 256: s * 512 + (t + 1) * 256],
                        start=(i == 0), stop=(i == nmm - 1),
                        tile_position=(0, 32 * j),
                    )
                    i += 1
            res_state["count"] += 1
            if res_state["count"] == 8:
                m = res_state["nflush"]
                nc.vector.tensor_copy(
                    res_state["rs"][:, m * 512:(m + 1) * 512], res_state["tile"][:]
                )
                res_state["count"] = 0
                res_state["tile"] = None
                res_state["nflush"] += 1
                if res_state["nflush"] == 4:
                    division_batch()

    def division_batch():
        b0 = res_state["first_g"] // N
        rs = res_state["rs"]
        scr = drs.tile([N, 768], FP, name="scr", tag="scr")
        rsv = rs[:].rearrange("(j p2) (m h q) -> j p2 m h q", j=4, m=4, h=2)
        sw = scr[:].rearrange("(m h j) (r q) -> j m h r q", m=4, h=2, r=3)
        for r in (0, 1, 2):
            nc.sync.dma_start(out=sw[:, :, :, r, :], in_=rsv[:, r, :, :, :])
        dn = dpool.tile([N, 768], FP, name="dn", tag="dn")
        nc.sync.dma_start(out=dn[:], in_=scr[:])
        dnv = dn[:].rearrange("p (r q) -> p r q", r=3)
        num = dpool.tile([N, BS], FP, name="num", tag="num")
        den = dpool.tile([N, BS], FP, name="den", tag="den")
        nc.vector.tensor_add(num[:], dnv[:, 1, :], dnv[:, 2, :])
        nc.vector.reciprocal_approx_fast(out=den[:], in_=dnv[:, 0, :])
        ov = dpool.tile([N, BS], FP, name="ov", tag="ov")
        nc.vector.tensor_mul(ov[:], num[:], den[:])
        nc.vector.tensor_scalar(
            out=ov[:], in0=ov[:], scalar1=sc["wvo"], scalar2=sc["bvo"] + sc["bo"],
            op0=ALU.mult, op1=ALU.add,
        )
        nc.sync.dma_start(out=out_t[:, b0 * BS:(b0 + 1) * BS], in_=ov[:])
        res_state["nflush"] = 0
        res_state["rs"] = None

    # --- main loop --------------------------------------------------------------
    pendings = []
    cur_qk = None
    vcur = [None]
    kcur = [None]
    NGRP = (PAIRS + GROUP - 1) // GROUP
    for g in range(NGRP):
        cls = group_class(g)
        p0 = g * GROUP
        npair = min(GROUP, PAIRS - p0)
        width = npair * 512
        members = []
        st = None
        sc16 = None
        for s in range(npair):
            p = p0 + s
            b, n = divmod(p, N)
            if n == 0:
                vcur[0], kcur[0] = load_vz(b)
            if p % SPD == 0:
                cur_qk = load_qz(p // SPD)
            sl = (p % SPD) * PW
            if cls == "C":
                if s == 0:
                    sc16 = spool.tile([128, GROUP * 512], F16, name="sc16", tag="sc16")
                qb = qpool.tile([128, 256], F16, name="qb", tag="qb")
                nc.gpsimd.partition_broadcast(qb[:], cur_qk[:][0:1, sl:sl + 256])
                for t in (0, 1):
                    nc.vector.tensor_scalar(
                        out=sc16[:, s * 512 + t * 256: s * 512 + (t + 1) * 256],
                        in0=qb[:], scalar1=kcur[0][:][:, t, n:n + 1],
                        scalar2=None, op0=ALU.mult, op1=ALU.bypass,
                    )
            else:
                if s == 0:
                    st = ps_stage.tile([128, GROUP * 512], FP, name="st", tag="st")
                nc.tensor.matmul(
                    st[:, s * 512:(s + 1) * 512],
                    lhsT=cur_qk[:][:, sl + 512: sl + PW],
                    rhs=cur_qk[:][:, sl: sl + 512],
                    start=True, stop=True,
                    tile_position=(0, 0),
                )
            members.append((s, b, n, vcur[0]))

        if cls == "B":
            a16 = spool.tile([128, GROUP * 512], I16, name="a16", tag="a16")
            b16 = spool.tile([128, GROUP * 512], I16, name="b16", tag="b16")
            nc.vector.tensor_scalar(
                out=a16[:, 0:width], in0=st[:][:, 0:width],
                scalar1=SCHR_SCALE, scalar2=SCHR_B1, op0=ALU.mult, op1=ALU.add,
            )
            nc.vector.tensor_scalar(
                out=b16[:, 0:width], in0=st[:][:, 0:width],
                scalar1=SCHR_SCALE, scalar2=SCHR_B2, op0=ALU.mult, op1=ALU.add,
            )
            rhs_tiles = [(a16[:].bitcast(F16), 0), (b16[:].bitcast(F16), 3)]
        else:
            e = epool.tile([128, GROUP * 512], F16, name="e", tag="e")
            src = sc16[:] if cls == "C" else st[:]
            nc.scalar.activation(e[:][:, 0:width], src[:, 0:width], AF.Exp)
            rhs_tiles = [(e[:], 0)]
        pendings.append((rhs_tiles, members))
        if len(pendings) > 3:
            emit_reduces(pendings.pop(0))
    for pend in pendings:
        emit_reduces(pend)
    assert res_state["count"] == 0 and res_state["nflush"] == 0, (
        "pair count must be a multiple of 32 (one block per division batch)"
    )


_CACHE: dict = {}


def _get_nc(sc, reps: int = 1) -> bass.Bass:
    key = (tuple(sorted(sc.items())), reps)
    if key not in _CACHE:
        _CACHE[key] = build_kernel_module(sc, reps)
    return _CACHE[key]


def make_in_maps(query, key, value, in_proj_w, in_proj_b, out_proj_w, out_proj_b):
    q = np.ascontiguousarray(np.asarray(query, dtype=np.float32).reshape(L, N))
    k = np.ascontiguousarray(np.asarray(key, dtype=np.float32).reshape(L, N))
    vv = np.ascontiguousarray(np.asarray(value, dtype=np.float32).reshape(L, N))
    wq, wk, wv = [float(x) for x in np.asarray(in_proj_w, dtype=np.float32).reshape(3)]
    bq, bk, bv = [float(x) for x in np.asarray(in_proj_b, dtype=np.float32).reshape(3)]
    wo = float(np.asarray(out_proj_w, dtype=np.float32).reshape(1)[0])
    bo = float(np.asarray(out_proj_b, dtype=np.float32).reshape(1)[0])
    sc = {"wvo": float(np.float32(wo) * np.float32(wv)),
          "bvo": float(np.float32(wo) * np.float32(bv)), "bo": bo}

    q16 = (q * np.float32(wq) + np.float32(bq)).astype(np.float16)
    k16 = (k * np.float32(wk) + np.float32(bk)).astype(np.float16)
    vhi = vv.astype(np.float16)
    vlo = (vv - vhi.astype(np.float32)).astype(np.float16)

    p = np.arange(PAIRS)
    b, n = p // N, p % N
    ar = np.arange(256)
    ar128 = np.arange(128)

    in_maps = []
    for c in range(NCORES):
        sl = slice(c * LS, (c + 1) * LS)
        qc = np.ascontiguousarray(q16[sl].T)   # [N, LS]
        kc = np.ascontiguousarray(k16[sl].T)
        qrows = qc[n[:, None], (b * BS)[:, None] + ar]            # [512, 256]
        qzc = np.zeros((PAIRS, 2, PW), np.float16)
        qzc[:, 0, 0:256] = qrows
        qzc[:, 1, 256:512] = qrows
        for t in (0, 1):
            qzc[:, t, 512:640] = kc[n[:, None], (b * BS + t * 128)[:, None] + ar128]
        # vz[b, p, (t, n, c6)]: cols 0:3 = (1, vhi, vlo), 3:6 = sqrt2 * same
        vzc = np.empty((BPC, 128, 2, N, 6), np.float16)
        vzc[:, :, :, :, 0] = 1.0
        vzc[:, :, :, :, 1] = vhi[sl].reshape(BPC, 2, 128, N).transpose(0, 2, 1, 3)
        vzc[:, :, :, :, 2] = vlo[sl].reshape(BPC, 2, 128, N).transpose(0, 2, 1, 3)
        vzc[:, :, :, :, 3:6] = (
            vzc[:, :, :, :, 0:3].astype(np.float32) * np.float32(SQRT2)
        ).astype(np.float16)
        # kz[b, p, (t, n)] = k'[b*256 + t*128 + p, n]
        kzc = np.ascontiguousarray(
            k16[sl].astype(np.float32).reshape(BPC, 2, 128, N).transpose(0, 2, 1, 3)
        )
        in_maps.append({
            "qz": np.ascontiguousarray(qzc),
            "vz": np.ascontiguousarray(vzc.reshape(BPC, 128, 2 * N * 6)),
            "kz": np.ascontiguousarray(kzc.reshape(BPC, 128, 2 * N)),
        })
    return in_maps, sc


def run(in_maps, sc, **kwargs):
    return run_bass_kernel_spmd(_get_nc(sc), in_maps, list(range(NCORES)), **kwargs)


def assemble(results) -> np.ndarray:
    outs = [np.asarray(results[c]["out_t"], dtype=np.float32).T for c in range(NCORES)]
    return np.ascontiguousarray(np.concatenate(outs, axis=0)).reshape(L, N, 1)


def kernel(query, key, value, in_proj_w, in_proj_b, out_proj_w, out_proj_b):
    in_maps, sc = make_in_maps(
        query, key, value, in_proj_w, in_proj_b, out_proj_w, out_proj_b
    )
    res = run(in_maps, sc)
    return assemble(res.results)
